# revision 50
# baseline (speedup 1.0000x reference)
"""Causal multi-head attention on 8 trn2 NeuronCores.

Problem: B=2, S=2048, D=2048, H=16 (HD=128), fp32 in/out.
Sharding: tensor-parallel over heads - core c owns heads {2c, 2c+1} for both
batches. Each core computes its Q/K/V projections, attention for its 4
(batch, head) pairs, and a partial output projection over its head slice.
The host sums the 8 partial outputs (transposing [B,D,S] -> [B,S,D]) and
adds the output bias.

Operands are bf16 in SBUF (fp32 PSUM accumulation) except the first NF8=6
of 16 contraction chunks of the Q/K projections, which run as fp8e4
DoubleRow matmuls (2 chunks per instruction at 2x PE rate). The fp8 logit
noise costs ~1.6e-2 max-norm rel err (gate 2e-2); the numpy sim in the
transcript tracked hardware within 1%, and any fp8 on the V/ctx/Wo path
blows the budget (heavy-tailed ctx), so everything else stays bf16.

Device algorithm (per core):
  Phase A: stream X^T (bf16 at 8x scale, plus an fp8 copy of the first NF8
           chunks) in 2KB-line DMAs; compute Q^T/K^T (head-dim on
           partitions, 512x-scaled weights, fp8-DR head + bf16 tail) and V
           (tokens on partitions, Wv/8 so V is natural scale); all
           SBUF-resident. V gets a ones-column appended ([V | 1]).
  Phase B: per (b, qb of 512 queries): score tiles S^T = K^T_chunk.T @ Q^T
           (k on partitions; the 4096^2 scale folds into the exp scale).
           Diagonal-block tiles only compute/exp the valid column range
           [t_loc*128, 512) and Pool-mask the single [128,128] diagonal
           chunk. Then per 128-query chunk i: ctx_ext[q, 0:129] =
           sum_j E_chunk(j).T @ [V|1] accumulated in PSUM - column 128 is
           the softmax denominator for free. A [128,1] DVE reciprocal + DVE
           per-partition-scalar multiply normalizes ctx into bf16, and a PE
           transpose flips it to [hd, q].
  Phase C: out^T tiles = sum_h Wo_chunk.T @ ctx^T, staged to fp16 and
           written as OUT[b, D, S]; the host sums the 8 fp16 partials in
           fp32 (fp16 OUT halves the HBM write traffic; fp32 OUT saturated
           the DMA queues at the tail).

  ACT exp (~640ns/tile) is slower than the 213ns score matmul, so score ops
  are WOVEN into the rest of the PE stream at one per ~560ns of PE time
  (660ns during phase A, where exps share the psQK ring with the
  projection chains and a faster pace fills it) with a 2-group lookahead;
  the first two groups' scores hide inside phase A's batch-1 projections.
  Batch 1's groups run qb-descending so the run ends on the smallest
  group; its AV interleaves into the previous group's out-projection, and
  the bare final epilogue closes the attention PSUM pools and reruns from
  a fresh 6-bank ring with copies split across ACT+DVE halves, so the
  ~610ns bank-free latency pipelines behind the 462ns matmul pairs.
  No max-subtraction is needed: scores are O(8) for this problem so exp
  cannot overflow, and softmax is shift-invariant.
"""

import os
from collections import deque
from contextlib import ExitStack

import numpy as np
import ml_dtypes

import concourse.bacc as bacc
import concourse.tile as tile
from concourse import mybir
from concourse.bass_utils import run_bass_kernel_spmd

BF16 = ml_dtypes.bfloat16
F8 = ml_dtypes.float8_e4m3
F8MAX = 240.0


def _install_neff_cache():
    """Cache compiled NEFFs on disk keyed by BIR content hash.

    Purely a compile-time memo: identical BIR -> identical NEFF, so repeat
    runs skip the multi-minute neuronxcc compile. No effect on execution.
    """
    import hashlib
    import shutil

    import concourse.bass2jax as _b2j
    import concourse.bass_utils as _bu

    if getattr(_bu, "_neff_cache_installed", False):
        return
    cache_dir = os.environ.get("NEFF_CACHE_DIR", "/tmp/neff_cache")
    orig = _bu.compile_bir_kernel

    def cached(bir_json, tmpdir, neff_name="file.neff"):
        try:
            os.makedirs(cache_dir, exist_ok=True)
            key = hashlib.sha256(bir_json).hexdigest()[:24]
            cpath = os.path.join(cache_dir, key + ".neff")
            dst = os.path.join(tmpdir, neff_name)
            if os.path.exists(cpath):
                shutil.copy(cpath, dst)
                return dst
            out = orig(bir_json, tmpdir, neff_name)
            shutil.copy(out, cpath)
            return out
        except OSError:
            return orig(bir_json, tmpdir, neff_name)

    _bu.compile_bir_kernel = cached
    _b2j.compile_bir_kernel = cached
    _bu._neff_cache_installed = True


_install_neff_cache()

B, S, D, H = 2, 2048, 2048, 16
HD = D // H          # 128
NCORES = 8
HPC = H // NCORES    # heads per core = 2
M = HPC * HD         # 256 output columns per core per projection
T = B * S            # 4096 total token rows
KO = D // 128        # 16 contraction chunks
NPAIR = T // 1024    # 4 phase-A token pairs of 1024
QB = S // 512        # 4 query blocks per batch
SC = S // 128        # 16 key chunks per sequence
HD1 = HD + 1         # V with ones column
SCALE = 1.0 / float(np.sqrt(HD))
# Q/K projections: the first NF8 contraction chunks run as fp8e4 DoubleRow
# matmuls (2 chunks per instruction at 2x rate). Host pre-scales X by 8 and
# Wq/Wk by 512 (exact powers of 2) so the fp8 and bf16 partial products
# accumulate at one consistent 4096x scale; exp() divides it back out.
# NF8=6 keeps the extra logit noise at ~1.6e-2 rel err (budget 2e-2);
# hardware matched the numpy fp8 sim within 1% at NF8=4 (1.281 vs 1.293e-2).
NF8 = 6
KOB = KO - NF8       # bf16 contraction chunks of Q/K
QKSC = 8.0 * 512.0
ESCALE = SCALE / (QKSC * QKSC)
# V projection: first NF8V chunks fp8-DR, reusing the same fp8 X tiles.
# Wv carries 64x (bf16 part too); the PSUM->SBUF copy divides by 8*64=512.
# Sim: nf8v=2 leaves rel err at 1.60e-2 (nf8v=4 would hit 1.99e-2).
NF8V = 2
KOBV = KO - NF8V     # bf16 contraction chunks of V

_built = {}


def _build(with_bias):
    f32 = mybir.dt.float32
    f16 = mybir.dt.float16
    bf16 = mybir.dt.bfloat16
    AF = mybir.ActivationFunctionType

    f8 = mybir.dt.float8e4
    DR = mybir.MatmulPerfMode.DoubleRow

    nc = bacc.Bacc(None, target_bir_lowering=False)

    # ---- per-core DRAM parameters (host supplies per-core shards) ----
    # XT[p, pair, ko, t] = 8 * x[pair*1024 + t, ko*128 + p]
    xt_p = nc.declare_dram_parameter("XT", [128, NPAIR, KO, 1024], bf16, False)
    # fp8 copy of the first NF8 chunks (same 8x scale) for the Q/K DR part
    xt8_p = nc.declare_dram_parameter(
        "XT8", [128, NPAIR, NF8, 1024], f8, False
    )
    # WqT/WkT[p, ko, m] = 512 * W[rows0 + m, (ko+NF8)*128 + p]  (bf16 part)
    wqt_p = nc.declare_dram_parameter("WQT", [128, KOB, M], bf16, False)
    wkt_p = nc.declare_dram_parameter("WKT", [128, KOB, M], bf16, False)
    # fp8 Wq/Wk chunks 0..NF8-1 at the same 512x scale
    wqt8_p = nc.declare_dram_parameter("WQT8", [128, NF8, M], f8, False)
    wkt8_p = nc.declare_dram_parameter("WKT8", [128, NF8, M], f8, False)
    # WvT at natural/8 scale so V comes out at natural scale (X carries 8x)
    wvt_p = nc.declare_dram_parameter("WVT", [128, KO, M], bf16, False)
    # WOT[p, h, oc, j] = Wo[oc*128 + j, rows0 + h*128 + p]
    wot_p = nc.declare_dram_parameter("WOT", [128, HPC, KO, 128], bf16, False)
    bias_p = nc.declare_dram_parameter("BIAS", [1, 3, M], bf16, False)
    mask_p = nc.declare_dram_parameter("MASK", [128, 128], bf16, False)
    iden_p = nc.declare_dram_parameter("IDEN", [128, 128], bf16, False)
    ones_p = nc.declare_dram_parameter("ONES", [128, 512], bf16, False)
    # fp16 partial outputs: halves HBM write traffic vs fp32, and the final
    # group's OUT burst no longer saturates the DMA queues at the tail
    out_p = nc.declare_dram_parameter("OUT", [B, D, S], f16, True)

    # batch 1 runs qb descending so the run ends on the smallest group
    # (1, 0): 8 exps and a 10-op AV keep the tail dependency chain short
    groups = [(0, 0), (0, 1), (0, 2), (0, 3), (1, 3), (1, 2), (1, 1), (1, 0)]
    SPACING = 560.0

    with tile.TileContext(nc) as tc:
        with (
            tc.tile_pool(name="persist", bufs=1) as persist,
            tc.tile_pool(name="bconst", bufs=1) as bconst,
            tc.tile_pool(name="epool", bufs=64) as epool,
        ):
            qt_res = persist.tile([128, B, HPC, S], bf16)
            kt_res = persist.tile([128, B, HPC, S], bf16)
            v_res = persist.tile([128, B, HPC, SC, HD1], bf16)
            # ones column of [V | 1]; disjoint from the phase-A V writes
            nc.vector.memset(v_res[:, :, :, :, HD:HD1], 1.0)

            # phase-B/C constants (DMAs queued below, after pair-0's X)
            masks = bconst.tile([128, 128], bf16, tag="masks")
            wot = bconst.tile([128, HPC, KO, 128], bf16, tag="wot")
            iden = bconst.tile([128, 128], bf16, tag="iden")

            def make_score_op(pool, tag, b, qb, t, h, out_list):
                def fn():
                    pss = pool.tile([128, 512], f32, tag=tag, name="pss")
                    e = epool.tile([128, 512], bf16, tag="e", name="e")
                    lhsT = kt_res[:, b, h, t * 128 : (t + 1) * 128]
                    t_loc = t - 4 * qb
                    if t_loc < 0:
                        # fully below the diagonal: whole tile is valid
                        nc.tensor.matmul(
                            pss,
                            lhsT=lhsT,
                            rhs=qt_res[:, b, h, qb * 512 : (qb + 1) * 512],
                            start=True,
                            stop=True,
                        )
                        nc.scalar.activation(e, pss, AF.Exp, scale=ESCALE)
                    else:
                        # diagonal-block tile: queries < t*128 are masked, so
                        # only compute cols [t_loc*128, 512). AV(i, j) reads
                        # es[j] col-chunk i only for i >= t_loc, so the
                        # unwritten low columns are never consumed.
                        c0 = t_loc * 128
                        nc.tensor.matmul(
                            pss[:, c0:512],
                            lhsT=lhsT,
                            rhs=qt_res[
                                :, b, h, qb * 512 + c0 : (qb + 1) * 512
                            ],
                            start=True,
                            stop=True,
                        )
                        nc.scalar.activation(
                            e[:, c0:512], pss[:, c0:512], AF.Exp, scale=ESCALE
                        )
                        # only the [128,128] chunk ON the diagonal needs the
                        # triangular mask; it runs on the idle Pool engine
                        nc.gpsimd.tensor_mul(
                            e[:, c0 : c0 + 128], e[:, c0 : c0 + 128], masks
                        )
                    out_list.append(e)

                return fn

            def weave(pe_ops, queue, acc, spacing=SPACING):
                """Emit pe_ops, inserting one queued score per `spacing` ns
                of accumulated PE time. Returns the leftover accum."""
                for cost, fn in pe_ops:
                    while queue and acc >= spacing:
                        queue.popleft()[1]()
                        acc -= spacing
                    fn()
                    acc += cost
                return acc

            def queue_scores(queue, pool, tag, gi, es_by):
                es_by[gi] = []
                b, qb = groups[gi]
                for t in range(4 * (qb + 1)):
                    for h in range(HPC):
                        queue.append(
                            (gi, make_score_op(pool, tag, b, qb, t, h, es_by[gi]))
                        )

            es_by = {}
            queue = deque()
            acc = 0.0

            # ---------------- Phase A: projections ----------------
            with (
                tc.tile_pool(name="wqkv", bufs=1) as wpool,
                tc.tile_pool(name="xs", bufs=3) as xpool,
                tc.tile_pool(name="x8s", bufs=3) as x8pool,
                tc.tile_pool(name="psQK", bufs=6, space="PSUM") as psQK,
                tc.tile_pool(name="psV", bufs=2, space="PSUM") as psV,
            ):
                wq = wpool.tile([128, KOB, M], bf16, tag="wq")
                wk = wpool.tile([128, KOB, M], bf16, tag="wk")
                wq8 = wpool.tile([128, NF8, M], f8, tag="wq8")
                wk8 = wpool.tile([128, NF8, M], f8, tag="wk8")
                wv = wpool.tile([128, KO, M], bf16, tag="wv")
                if with_bias:
                    bias = wpool.tile([1, 3, M], bf16, tag="bias")
                    ones_t = wpool.tile([128, 512], bf16, tag="ones_a")
                    ones = ones_t[0:1, :]

                def qk_bias_mm(ps, bi, h):
                    nc.tensor.matmul(
                        ps,
                        lhsT=bias[:, bi, h * HD : (h + 1) * HD],
                        rhs=ones,
                        start=False,
                        stop=True,
                    )

                def pair_dma(pair, xt_h, xt8):
                    if pair == 0:
                        # JIT startup: the fp8 Wq + X chunks are tiny and
                        # land first, unlocking the DR waves ~1.5us in; then
                        # bf16 wq 2-ko chunks interleave with X chunks 4..15.
                        # The V-only X chunks 0..3 and phase-B constants last.
                        nc.scalar.dma_start(wq8, wqt8_p[:])
                        for k2 in range(NF8 // 2):
                            nc.sync.dma_start(
                                xt8[:, 2 * k2 : 2 * k2 + 2],
                                xt8_p[:, 0, 2 * k2 : 2 * k2 + 2],
                            )
                        for g in range(KOB // 2):
                            gko = NF8 + 2 * g
                            (nc.scalar if g % 2 else nc.sync).dma_start(
                                wq[:, 2 * g : 2 * g + 2],
                                wqt_p[:, 2 * g : 2 * g + 2],
                            )
                            (nc.sync if g % 2 else nc.scalar).dma_start(
                                xt_h[gko // 8][:, gko % 8 : gko % 8 + 2],
                                xt_p[:, 0, gko : gko + 2],
                            )
                        nc.scalar.dma_start(wk8, wkt8_p[:])
                        nc.sync.dma_start(wk, wkt_p[:])
                        nc.sync.dma_start(xt_h[0][:, 0:NF8], xt_p[:, 0, 0:NF8])
                        nc.sync.dma_start(wv, wvt_p[:])
                        nc.sync.dma_start(masks, mask_p[:])
                        nc.sync.dma_start(wot, wot_p[:])
                        nc.sync.dma_start(iden, iden_p[:])
                        if with_bias:
                            nc.sync.dma_start(bias, bias_p[:])
                            nc.sync.dma_start(ones_t, ones_p[:])
                    else:
                        nc.sync.dma_start(xt8, xt8_p[:, pair])
                        for half in range(2):
                            nc.sync.dma_start(
                                xt_h[half],
                                xt_p[:, pair, half * 8 : half * 8 + 8],
                            )

                def pair_ops(pair, xt_h, xt8):
                    """Projection compute for one 1024-token pair, as lists
                    of (cost_ns, fn) ops keyed by ('q'|'k'|'v', sub)."""
                    b = pair // 2
                    state = {}

                    def xt_at(ko, sub):
                        return xt_h[ko // 8][
                            :, ko % 8, sub * 512 : (sub + 1) * 512
                        ]

                    def qk_ops(sub, s0, w8, wt, dst, bi):
                        ops = []
                        for h in range(HPC):
                            for k2 in range(NF8 // 2):
                                def fn(sub=sub, w8=w8, bi=bi, h=h, k2=k2):
                                    key = (sub, bi, h)
                                    if k2 == 0:
                                        state[key] = psQK.tile(
                                            [128, 512], f32,
                                            tag="qk", name="psqk",
                                        )
                                    nc.tensor.matmul(
                                        state[key],
                                        lhsT=w8[
                                            :, 2 * k2 : 2 * k2 + 2,
                                            h * HD : (h + 1) * HD,
                                        ],
                                        rhs=xt8[
                                            :, 2 * k2 : 2 * k2 + 2,
                                            sub * 512 : (sub + 1) * 512,
                                        ],
                                        start=(k2 == 0),
                                        stop=False,
                                        perf_mode=DR,
                                    )
                                ops.append((231, fn))
                            for ko in range(KOB):
                                def fn(
                                    sub=sub, s0=s0, wt=wt, dst=dst,
                                    bi=bi, h=h, ko=ko,
                                ):
                                    ps = state[(sub, bi, h)]
                                    nc.tensor.matmul(
                                        ps,
                                        lhsT=wt[:, ko, h * HD : (h + 1) * HD],
                                        rhs=xt_at(ko + NF8, sub),
                                        start=False,
                                        stop=(ko == KOB - 1)
                                        and not with_bias,
                                    )
                                    if ko == KOB - 1:
                                        if with_bias:
                                            qk_bias_mm(ps, bi, h)
                                        nc.vector.tensor_copy(
                                            dst[:, b, h, s0 : s0 + 512], ps
                                        )
                                ops.append((213, fn))
                        return ops

                    res = {}
                    for sub in range(2):
                        s0 = (pair * 1024 + sub * 512) % S
                        res[("q", sub)] = qk_ops(sub, s0, wq8, wq, qt_res, 0)
                        res[("k", sub)] = qk_ops(sub, s0, wk8, wk, kt_res, 1)
                        vops = []
                        for tsub in range(4):
                            for ko in range(KO):
                                def fn(sub=sub, s0=s0, tsub=tsub, ko=ko):
                                    key = ("v", sub, tsub)
                                    if ko == 0:
                                        state[key] = psV.tile(
                                            [128, M], f32, tag="v", name="psv"
                                        )
                                    ps = state[key]
                                    nc.tensor.matmul(
                                        ps,
                                        lhsT=xt_at(ko, sub)[
                                            :, tsub * 128 : (tsub + 1) * 128
                                        ],
                                        rhs=wv[:, ko],
                                        start=(ko == 0),
                                        stop=(ko == KO - 1) and not with_bias,
                                    )
                                    if ko == KO - 1:
                                        if with_bias:
                                            nc.tensor.matmul(
                                                ps,
                                                lhsT=ones[:, :128],
                                                rhs=bias[:, 2],
                                                start=False,
                                                stop=True,
                                            )
                                        sc = (s0 + tsub * 128) // 128
                                        nc.vector.tensor_copy(
                                            v_res[:, b, :, sc, 0:HD],
                                            ps.rearrange(
                                                "p (h d) -> p h d", h=HPC
                                            ),
                                        )
                                vops.append((107, fn))
                        res[("v", sub)] = vops
                    return res

                def flat_ops(res):
                    out = []
                    for sub in range(2):
                        for kind in ("q", "k", "v"):
                            out += res[(kind, sub)]
                    return out

                def new_x_tiles():
                    xt_h = [
                        xpool.tile(
                            [128, KO // 2, 1024], bf16, tag="xt", name="xth"
                        )
                        for _ in range(2)
                    ]
                    xt8 = x8pool.tile([128, NF8, 1024], f8, tag="x8", name="x8")
                    return xt_h, xt8

                # pair 0: interleave the 4 Q chains (h, sub) wave by wave so
                # they start as soon as the first fp8/bf16 chunks land
                xt_h0, xt8_0 = new_x_tiles()
                pair_dma(0, xt_h0, xt8_0)
                ops0 = pair_ops(0, xt_h0, xt8_0)
                per = NF8 // 2 + KOB        # ops per (h, sub) Q chain
                for w in range(per):
                    for sub in range(2):
                        qsub = ops0[("q", sub)]
                        for h in range(HPC):
                            qsub[h * per + w][1]()
                # pair 0 K + V
                for key in (("k", 0), ("v", 0), ("k", 1), ("v", 1)):
                    for _, fn in ops0[key]:
                        fn()

                # pair 1: emitted bare
                xt_h1, xt8_1 = new_x_tiles()
                pair_dma(1, xt_h1, xt8_1)
                for _, fn in flat_ops(pair_ops(1, xt_h1, xt8_1)):
                    fn()

                # batch 0's Q/K/V are ready: weave groups 0+1's scores into
                # batch 1's projection compute (exps run during phase A)
                queue_scores(queue, psQK, "qk", 0, es_by)
                queue_scores(queue, psQK, "qk", 1, es_by)
                for pair in (2, 3):
                    xt_h, xt8 = new_x_tiles()
                    pair_dma(pair, xt_h, xt8)
                    # pace at >= the ~640ns exp service time: at 560 the ACT
                    # backlog fills the shared psQK ring and stalls the PE
                    acc = weave(
                        flat_ops(pair_ops(pair, xt_h, xt8)), queue, acc,
                        spacing=660.0,
                    )

            # ------------- Phase B + C: attention + out projection -------------
            with (
                tc.tile_pool(name="ctxn", bufs=12) as ctxn,
                tc.tile_pool(name="recp", bufs=12) as recp,
                tc.tile_pool(name="ctxT", bufs=2) as ctxTp,
                tc.tile_pool(name="ob", bufs=6) as obp,
            ):
                # attention-phase PSUM pools live in their own scope so the
                # bare final epilogue can reuse the banks as one deep ring
                ps_stack = ExitStack()
                psS = ps_stack.enter_context(
                    tc.tile_pool(name="psS", bufs=2, space="PSUM")
                )
                psC = ps_stack.enter_context(
                    tc.tile_pool(name="psC", bufs=3, space="PSUM")
                )
                psT = ps_stack.enter_context(
                    tc.tile_pool(name="psT", bufs=1, space="PSUM")
                )
                psO = ps_stack.enter_context(
                    tc.tile_pool(name="psO", bufs=2, space="PSUM")
                )
                def av_ops(b, qb, es, cns_out, tc_i=None):
                    """One op per k-chunk j of each 128-query chunk i; the
                    closing op of each i-chunk adds the DVE rec+normalize
                    (plus, for the last group, its transposes via tc_i)."""
                    ops = []
                    state = {}
                    for i in range(4):
                        qi = 4 * qb + i
                        for j in range(qi + 1):
                            def fn(i=i, j=j, qi=qi):
                                if j == 0:
                                    state[i] = [
                                        psC.tile(
                                            [128, 512], f32, tag="c", name="psc"
                                        )
                                        for _ in range(HPC)
                                    ]
                                pscs = state[i]
                                for h in range(HPC):
                                    nc.tensor.matmul(
                                        pscs[h][:, 0:HD1],
                                        lhsT=es[2 * j + h][
                                            :, i * 128 : (i + 1) * 128
                                        ],
                                        rhs=v_res[:, b, h, j, :],
                                        start=(j == 0),
                                        stop=(j == qi),
                                    )
                                if j == qi:
                                    cns_pair = []
                                    for h in range(HPC):
                                        rec = recp.tile(
                                            [128, 1], f32, tag="r", name="rec"
                                        )
                                        nc.vector.reciprocal(
                                            rec, pscs[h][:, HD:HD1]
                                        )
                                        cn = ctxn.tile(
                                            [128, 128], bf16, tag="cn", name="cn"
                                        )
                                        nc.vector.tensor_scalar_mul(
                                            cn, pscs[h][:, 0:HD], rec
                                        )
                                        cns_pair.append(cn)
                                    cns_out.extend(cns_pair)
                                    if tc_i is not None:
                                        tc_i(i, cns_pair)
                            ops.append((110, fn))
                    return ops

                def make_tc_t(ct):
                    """Per-chunk transposes for the final group, so the
                    epilogue is only the out projection."""
                    def tc_i(i, cns_pair):
                        for h in range(HPC):
                            pst = psT.tile([128, 512], bf16, tag="t", name="pst")
                            nc.tensor.transpose(pst[:, 0:128], cns_pair[h], iden)
                            nc.vector.tensor_copy(
                                ct[:, h, i * 128 : (i + 1) * 128], pst[:, 0:128]
                            )
                    return tc_i

                def tc_ops(b, qb, cns, ct, skip_T=False, alt_q=False,
                           cp=None, pool_o=None):
                    """Transpose normalized ctx, then the out projection.
                    Output tiles are paired into one DMA per 256 rows."""
                    ops = []
                    if not skip_T:
                        for i in range(4):
                            for h in range(HPC):
                                def fn(i=i, h=h):
                                    pst = psT.tile(
                                        [128, 512], bf16, tag="t", name="pst"
                                    )
                                    nc.tensor.transpose(
                                        pst[:, 0:128], cns[2 * i + h], iden
                                    )
                                    nc.vector.tensor_copy(
                                        ct[:, h, i * 128 : (i + 1) * 128],
                                        pst[:, 0:128],
                                    )
                                ops.append((110, fn))
                    state = {}
                    for oc in range(KO):
                        def fn(oc=oc):
                            pso = (pool_o or psO).tile(
                                [128, 512], f32, tag="o", name="pso"
                            )
                            for h in range(HPC):
                                nc.tensor.matmul(
                                    pso,
                                    lhsT=wot[:, h, oc],
                                    rhs=ct[:, h, :],
                                    start=(h == 0),
                                    stop=(h == HPC - 1),
                                )
                            def ccopy(dst, src, oc=oc):
                                eng = cp[oc % len(cp)] if cp else "dve"
                                if eng == "split":
                                    # halves on both engines concurrently:
                                    # frees the PSUM bank in ~400ns so the
                                    # epilogue stays PE-bound
                                    nc.scalar.activation(
                                        dst[:, 0:256], src[:, 0:256], AF.Copy
                                    )
                                    nc.vector.tensor_copy(
                                        dst[:, 256:512], src[:, 256:512]
                                    )
                                elif eng == "act":
                                    nc.scalar.activation(dst, src, AF.Copy)
                                else:
                                    nc.vector.tensor_copy(dst, src)
                            if oc % 2 == 0:
                                state["ob"] = obp.tile(
                                    [128, 2, 512], f16, tag="ob", name="ob"
                                )
                                ccopy(state["ob"][:, 0], pso)
                            else:
                                ob = state["ob"]
                                ccopy(ob[:, 1], pso)
                                eng = (
                                    nc.scalar
                                    if alt_q and (oc // 2) % 2
                                    else nc.sync
                                )
                                eng.dma_start(
                                    out_p[
                                        b,
                                        (oc - 1) * 128 : (oc + 1) * 128,
                                        qb * 512 : (qb + 1) * 512,
                                    ].rearrange("(u p) s -> p u s", u=2),
                                    ob,
                                )
                        ops.append((430, fn))
                    return ops

                prev = None
                last_tc = None
                for gi, (b, qb) in enumerate(groups):
                    if gi + 2 < len(groups):
                        queue_scores(queue, psS, "s", gi + 2, es_by)
                    last = gi == len(groups) - 1
                    # late groups: exps are mostly done so ACT has slack,
                    # while DVE carries the AV normalize chain -- stage the
                    # out tiles on ACT to keep DVE off the critical path.
                    # (Measured dead ends: split-halves for gi>=4 or
                    # act/dve+split mixes for gi>=5 both regress 7-15us.)
                    cp = ["act"] if gi >= 5 else None
                    pre_ops = [] if prev is None else tc_ops(*prev, cp=cp)
                    cns = []
                    ct = ctxTp.tile([128, HPC, 512], bf16, tag="ct", name="ct")
                    if not last:
                        acc = weave(pre_ops, queue, acc)
                        # barrier: scores(g) all emitted before AV(g)
                        while queue and queue[0][0] <= gi:
                            queue.popleft()[1]()
                            acc = 0.0
                        acc = weave(
                            av_ops(b, qb, es_by[gi], cns, tc_i=None),
                            queue, acc,
                        )
                        prev = (b, qb, cns, ct)
                    else:
                        # final group: drain the queue inside the first part
                        # of prev's out-projection, then interleave the tiny
                        # AV so its DVE-latency chain hides under PE work
                        # and the epilogue can start immediately after
                        n_head = min(12, len(pre_ops))
                        acc = weave(pre_ops[:n_head], queue, acc)
                        while queue and queue[0][0] <= gi:
                            queue.popleft()[1]()
                            acc = 0.0
                        avl = av_ops(
                            b, qb, es_by[gi], cns, tc_i=make_tc_t(ct)
                        )
                        rest = pre_ops[n_head:]
                        merged = []
                        ai = 0
                        for k, op in enumerate(rest):
                            merged.append(op)
                            want = (k + 1) * len(avl) // max(len(rest), 1)
                            while ai < min(want, len(avl)):
                                merged.append(avl[ai])
                                ai += 1
                        merged.extend(avl[ai:])
                        for _, fn in merged:
                            fn()
                        last_tc = (b, qb, cns, ct)
                        prev = None
                    del es_by[gi]
                if prev is not None:
                    for _, fn in tc_ops(*prev):
                        fn()
                if last_tc is not None:
                    ps_stack.close()
                    with tc.tile_pool(
                        name="psE", bufs=6, space="PSUM"
                    ) as psE:
                        for _, fn in tc_ops(
                            *last_tc, skip_T=True, alt_q=False,
                            cp=["split"], pool_o=psE,
                        ):
                            fn()
                else:
                    ps_stack.close()

    nc.finalize()
    return nc


def _get_nc(with_bias=False):
    if with_bias not in _built:
        _built[with_bias] = _build(with_bias)
    return _built[with_bias]


def kernel(hidden_states, attention_mask, Wq, bq, Wk, bk, Wv, bv, Wo, bo):
    hidden_states = np.asarray(hidden_states, dtype=np.float32)
    Wq, Wk, Wv, Wo = (np.asarray(w, dtype=np.float32) for w in (Wq, Wk, Wv, Wo))
    bq, bk, bv, bo = (np.asarray(v, dtype=np.float32) for v in (bq, bk, bv, bo))

    with_bias = bool(np.any(bq) or np.any(bk) or np.any(bv))

    x8 = hidden_states.reshape(T, D) * 8.0
    # XT[p, pair, ko, t] = 8 * x[pair*1024 + t, ko*128 + p]
    xt = np.ascontiguousarray(
        x8.reshape(NPAIR, 1024, KO, 128).transpose(3, 0, 2, 1)
    ).astype(BF16)
    # fp8 copy of chunks 0..NF8-1 at the same 8x scale (Q/K DoubleRow part)
    xt8 = np.ascontiguousarray(
        np.clip(x8[:, : NF8 * 128], -F8MAX, F8MAX)
        .reshape(NPAIR, 1024, NF8, 128)
        .transpose(3, 0, 2, 1)
    ).astype(F8)

    # causal 0/1 mask for the single [128,128] chunk on the diagonal:
    # mask[p, f] = p <= f
    mask = (np.arange(128)[:, None] <= np.arange(128)[None, :]).astype(BF16)
    iden = np.eye(128, dtype=BF16)
    ones = np.ones((128, 512), dtype=BF16)

    in_maps = []
    d8 = NF8 * 128
    for c in range(NCORES):
        rows = slice(c * M, (c + 1) * M)
        # bf16 W*T chunks NF8.. at 512x scale (matches the fp8 part's scale)
        wqt = np.ascontiguousarray(
            (Wq[rows, d8:].T * 512.0).reshape(KOB, 128, M).transpose(1, 0, 2)
        ).astype(BF16)
        wkt = np.ascontiguousarray(
            (Wk[rows, d8:].T * 512.0).reshape(KOB, 128, M).transpose(1, 0, 2)
        ).astype(BF16)
        wqt8 = np.ascontiguousarray(
            np.clip(Wq[rows, :d8].T * 512.0, -F8MAX, F8MAX)
            .reshape(NF8, 128, M)
            .transpose(1, 0, 2)
        ).astype(F8)
        wkt8 = np.ascontiguousarray(
            np.clip(Wk[rows, :d8].T * 512.0, -F8MAX, F8MAX)
            .reshape(NF8, 128, M)
            .transpose(1, 0, 2)
        ).astype(F8)
        # Wv/8 cancels X's 8x so V lands at natural scale
        wvt = np.ascontiguousarray(
            (Wv[rows, :].T / 8.0).reshape(KO, 128, M).transpose(1, 0, 2)
        ).astype(BF16)
        # WOT[p, h, oc, j] = Wo[oc*128 + j, rows0 + h*128 + p]
        wot = np.ascontiguousarray(
            Wo[:, rows].reshape(KO, 128, HPC, 128).transpose(3, 2, 0, 1)
        ).astype(BF16)
        bias = np.stack(
            [bq[rows] * 4096.0, bk[rows] * 4096.0, bv[rows]]
        )[None].astype(BF16)
        in_maps.append(
            {
                "XT": xt,
                "XT8": xt8,
                "WQT": wqt,
                "WKT": wkt,
                "WQT8": wqt8,
                "WKT8": wkt8,
                "WVT": wvt,
                "WOT": wot,
                "BIAS": np.ascontiguousarray(bias),
                "MASK": mask,
                "IDEN": iden,
                "ONES": ones,
            }
        )

    res = run_bass_kernel_spmd(_get_nc(with_bias), in_maps, list(range(NCORES)))
    out = res.results[0]["OUT"].astype(np.float32)
    for c in range(1, NCORES):
        out += res.results[c]["OUT"].astype(np.float32)
    out = np.ascontiguousarray(out.transpose(0, 2, 1))
    out += bo
    return out



# revision 52
# speedup vs baseline: 1.1722x; 1.1722x over previous
"""Causal multi-head attention on 8 trn2 NeuronCores.

Problem: B=2, S=2048, D=2048, H=16 (HD=128), fp32 in/out.
Sharding: tensor-parallel over heads - core c owns heads {2c, 2c+1} for both
batches. Each core computes its Q/K/V projections, attention for its 4
(batch, head) pairs, and a partial output projection over its head slice.
The host sums the 8 partial outputs (transposing [B,D,S] -> [B,S,D]) and
adds the output bias.

Operands are bf16 in SBUF (fp32 PSUM accumulation) except the first NF8=6
of 16 contraction chunks of the Q/K projections, which run as fp8e4
DoubleRow matmuls (2 chunks per instruction at 2x PE rate). The fp8 logit
noise costs ~1.6e-2 max-norm rel err (gate 2e-2); the numpy sim in the
transcript tracked hardware within 1%, and any fp8 on the V/ctx/Wo path
blows the budget (heavy-tailed ctx), so everything else stays bf16.

Device algorithm (per core):
  Phase A: stream X^T (bf16 at 8x scale, plus an fp8 copy of the first NF8
           chunks) in 2KB-line DMAs; compute Q^T/K^T (head-dim on
           partitions, 512x-scaled weights, fp8-DR head + bf16 tail) and V
           (tokens on partitions, Wv/8 so V is natural scale); all
           SBUF-resident. V gets a ones-column appended ([V | 1]).
  Phase B: per (b, qb of 512 queries): score tiles S^T = K^T_chunk.T @ Q^T
           (k on partitions; the 4096^2 scale folds into the exp scale).
           Diagonal-block tiles only compute/exp the valid column range
           [t_loc*128, 512) and Pool-mask the single [128,128] diagonal
           chunk. Then per 128-query chunk i: ctx_ext[q, 0:129] =
           sum_j E_chunk(j).T @ [V|1] accumulated in PSUM - column 128 is
           the softmax denominator for free. A [128,1] DVE reciprocal + DVE
           per-partition-scalar multiply normalizes ctx into bf16, and a PE
           transpose flips it to [hd, q].
  Phase C: out^T tiles = sum_h Wo_chunk.T @ ctx^T, staged to fp16 and
           written as OUT[b, D, S]; the host sums the 8 fp16 partials in
           fp32 (fp16 OUT halves the HBM write traffic; fp32 OUT saturated
           the DMA queues at the tail).

  ACT exp (~640ns/tile) is slower than the 213ns score matmul, so score ops
  are WOVEN into the rest of the PE stream at one per ~560ns of PE time
  (660ns during phase A, where exps share the psQK ring with the
  projection chains and a faster pace fills it) with a 2-group lookahead;
  the first two groups' scores hide inside phase A's batch-1 projections.
  Batch 1's groups run qb-descending so the run ends on the smallest
  group; its AV interleaves into the previous group's out-projection, and
  the bare final epilogue closes the attention PSUM pools and reruns from
  a fresh 6-bank ring with copies split across ACT+DVE halves, so the
  ~610ns bank-free latency pipelines behind the 462ns matmul pairs.
  No max-subtraction is needed: scores are O(8) for this problem so exp
  cannot overflow, and softmax is shift-invariant.
"""

import os
from collections import deque
from contextlib import ExitStack

import numpy as np
import ml_dtypes

import concourse.bacc as bacc
import concourse.tile as tile
from concourse import mybir
from concourse.bass_utils import run_bass_kernel_spmd

BF16 = ml_dtypes.bfloat16
F8 = ml_dtypes.float8_e4m3
F8MAX = 240.0


def _install_neff_cache():
    """Cache compiled NEFFs on disk keyed by BIR content hash.

    Purely a compile-time memo: identical BIR -> identical NEFF, so repeat
    runs skip the multi-minute neuronxcc compile. No effect on execution.
    """
    import hashlib
    import shutil

    import concourse.bass2jax as _b2j
    import concourse.bass_utils as _bu

    if getattr(_bu, "_neff_cache_installed", False):
        return
    cache_dir = os.environ.get("NEFF_CACHE_DIR", "/tmp/neff_cache")
    orig = _bu.compile_bir_kernel

    def cached(bir_json, tmpdir, neff_name="file.neff"):
        try:
            os.makedirs(cache_dir, exist_ok=True)
            key = hashlib.sha256(bir_json).hexdigest()[:24]
            cpath = os.path.join(cache_dir, key + ".neff")
            dst = os.path.join(tmpdir, neff_name)
            if os.path.exists(cpath):
                shutil.copy(cpath, dst)
                return dst
            out = orig(bir_json, tmpdir, neff_name)
            shutil.copy(out, cpath)
            return out
        except OSError:
            return orig(bir_json, tmpdir, neff_name)

    _bu.compile_bir_kernel = cached
    _b2j.compile_bir_kernel = cached
    _bu._neff_cache_installed = True


_install_neff_cache()

B, S, D, H = 2, 2048, 2048, 16
HD = D // H          # 128
NCORES = 8
HPC = H // NCORES    # heads per core = 2
M = HPC * HD         # 256 output columns per core per projection
T = B * S            # 4096 total token rows
KO = D // 128        # 16 contraction chunks
NPAIR = T // 1024    # 4 phase-A token pairs of 1024
QB = S // 512        # 4 query blocks per batch
SC = S // 128        # 16 key chunks per sequence
HD1 = HD + 1         # V with ones column
SCALE = 1.0 / float(np.sqrt(HD))
# Q/K projections: the first NF8 contraction chunks run as fp8e4 DoubleRow
# matmuls (2 chunks per instruction at 2x rate). Host pre-scales X by 8 and
# Wq/Wk by 512 (exact powers of 2) so the fp8 and bf16 partial products
# accumulate at one consistent 4096x scale; exp() divides it back out.
# NF8=6 keeps the extra logit noise at ~1.6e-2 rel err (budget 2e-2);
# hardware matched the numpy fp8 sim within 1% at NF8=4 (1.281 vs 1.293e-2).
NF8 = 6
KOB = KO - NF8       # bf16 contraction chunks of Q/K
QKSC = 8.0 * 512.0
ESCALE = SCALE / (QKSC * QKSC)
# V projection: first NF8V chunks fp8-DR, reusing the same fp8 X tiles.
# Wv carries 64x (bf16 part too); the PSUM->SBUF copy divides by 8*64=512.
# Sim: nf8v=2 leaves rel err at 1.60e-2 (nf8v=4 would hit 1.99e-2).
NF8V = 2
KOBV = KO - NF8V     # bf16 contraction chunks of V

_built = {}


def _build(with_bias):
    f32 = mybir.dt.float32
    f16 = mybir.dt.float16
    bf16 = mybir.dt.bfloat16
    AF = mybir.ActivationFunctionType

    f8 = mybir.dt.float8e4
    DR = mybir.MatmulPerfMode.DoubleRow

    nc = bacc.Bacc(None, target_bir_lowering=False)

    # ---- per-core DRAM parameters (host supplies per-core shards) ----
    # XT[p, pair, ko, t] = 8 * x[pair*1024 + t, ko*128 + p]
    xt_p = nc.declare_dram_parameter("XT", [128, NPAIR, KO, 1024], bf16, False)
    # fp8 copy of the first NF8 chunks (same 8x scale) for the Q/K DR part
    xt8_p = nc.declare_dram_parameter(
        "XT8", [128, NPAIR, NF8, 1024], f8, False
    )
    # WqT/WkT[p, ko, m] = 512 * W[rows0 + m, (ko+NF8)*128 + p]  (bf16 part)
    wqt_p = nc.declare_dram_parameter("WQT", [128, KOB, M], bf16, False)
    wkt_p = nc.declare_dram_parameter("WKT", [128, KOB, M], bf16, False)
    # fp8 Wq/Wk chunks 0..NF8-1 at the same 512x scale
    wqt8_p = nc.declare_dram_parameter("WQT8", [128, NF8, M], f8, False)
    wkt8_p = nc.declare_dram_parameter("WKT8", [128, NF8, M], f8, False)
    # WvT at natural/8 scale so V comes out at natural scale (X carries 8x)
    wvt_p = nc.declare_dram_parameter("WVT", [128, KO, M], bf16, False)
    # WOT[p, h, oc, j] = Wo[oc*128 + j, rows0 + h*128 + p]
    wot_p = nc.declare_dram_parameter("WOT", [128, HPC, KO, 128], bf16, False)
    bias_p = nc.declare_dram_parameter("BIAS", [1, 3, M], bf16, False)
    mask_p = nc.declare_dram_parameter("MASK", [128, 128], bf16, False)
    iden_p = nc.declare_dram_parameter("IDEN", [128, 128], bf16, False)
    ones_p = nc.declare_dram_parameter("ONES", [128, 512], bf16, False)
    # fp16 partial outputs: halves HBM write traffic vs fp32, and the final
    # group's OUT burst no longer saturates the DMA queues at the tail
    out_p = nc.declare_dram_parameter("OUT", [B, D, S], f16, True)

    # batch 1 runs qb descending so the run ends on the smallest group
    # (1, 0): 8 exps and a 10-op AV keep the tail dependency chain short
    groups = [(0, 0), (0, 1), (0, 2), (0, 3), (1, 3), (1, 2), (1, 1), (1, 0)]
    SPACING = 560.0

    with tile.TileContext(nc) as tc:
        with (
            tc.tile_pool(name="persist", bufs=1) as persist,
            tc.tile_pool(name="bconst", bufs=1) as bconst,
            tc.tile_pool(name="epool", bufs=64) as epool,
        ):
            qt_res = persist.tile([128, B, HPC, S], bf16)
            kt_res = persist.tile([128, B, HPC, S], bf16)
            v_res = persist.tile([128, B, HPC, SC, HD1], bf16)
            # ones column of [V | 1]; disjoint from the phase-A V writes
            nc.vector.memset(v_res[:, :, :, :, HD:HD1], 1.0)

            # phase-B/C constants (DMAs queued below, after pair-0's X)
            masks = bconst.tile([128, 128], bf16, tag="masks")
            wot = bconst.tile([128, HPC, KO, 128], bf16, tag="wot")
            iden = bconst.tile([128, 128], bf16, tag="iden")

            def make_score_op(pool, tag, b, qb, t, h, out_list):
                def fn():
                    pss = pool.tile([128, 512], f32, tag=tag, name="pss")
                    e = epool.tile([128, 512], bf16, tag="e", name="e")
                    lhsT = kt_res[:, b, h, t * 128 : (t + 1) * 128]
                    t_loc = t - 4 * qb
                    if t_loc < 0:
                        # fully below the diagonal: whole tile is valid
                        nc.tensor.matmul(
                            pss,
                            lhsT=lhsT,
                            rhs=qt_res[:, b, h, qb * 512 : (qb + 1) * 512],
                            start=True,
                            stop=True,
                        )
                        nc.scalar.activation(e, pss, AF.Exp, scale=ESCALE)
                    else:
                        # diagonal-block tile: queries < t*128 are masked, so
                        # only compute cols [t_loc*128, 512). AV(i, j) reads
                        # es[j] col-chunk i only for i >= t_loc, so the
                        # unwritten low columns are never consumed.
                        c0 = t_loc * 128
                        nc.tensor.matmul(
                            pss[:, c0:512],
                            lhsT=lhsT,
                            rhs=qt_res[
                                :, b, h, qb * 512 + c0 : (qb + 1) * 512
                            ],
                            start=True,
                            stop=True,
                        )
                        nc.scalar.activation(
                            e[:, c0:512], pss[:, c0:512], AF.Exp, scale=ESCALE
                        )
                        # only the [128,128] chunk ON the diagonal needs the
                        # triangular mask; it runs on the idle Pool engine
                        nc.gpsimd.tensor_mul(
                            e[:, c0 : c0 + 128], e[:, c0 : c0 + 128], masks
                        )
                    out_list.append(e)

                return fn

            def weave(pe_ops, queue, acc, spacing=SPACING):
                """Emit pe_ops, inserting one queued score per `spacing` ns
                of accumulated PE time. Returns the leftover accum."""
                for cost, fn in pe_ops:
                    while queue and acc >= spacing:
                        queue.popleft()[1]()
                        acc -= spacing
                    fn()
                    acc += cost
                return acc

            def queue_scores(queue, pool, tag, gi, es_by):
                es_by[gi] = []
                b, qb = groups[gi]
                for t in range(4 * (qb + 1)):
                    for h in range(HPC):
                        queue.append(
                            (gi, make_score_op(pool, tag, b, qb, t, h, es_by[gi]))
                        )

            es_by = {}
            queue = deque()
            acc = 0.0

            # ---------------- Phase A: projections ----------------
            with (
                tc.tile_pool(name="wqkv", bufs=1) as wpool,
                tc.tile_pool(name="xs", bufs=3) as xpool,
                tc.tile_pool(name="x8s", bufs=3) as x8pool,
                tc.tile_pool(name="psQK", bufs=6, space="PSUM") as psQK,
                tc.tile_pool(name="psV", bufs=2, space="PSUM") as psV,
            ):
                wq = wpool.tile([128, KOB, M], bf16, tag="wq")
                wk = wpool.tile([128, KOB, M], bf16, tag="wk")
                wq8 = wpool.tile([128, NF8, M], f8, tag="wq8")
                wk8 = wpool.tile([128, NF8, M], f8, tag="wk8")
                wv = wpool.tile([128, KO, M], bf16, tag="wv")
                if with_bias:
                    bias = wpool.tile([1, 3, M], bf16, tag="bias")
                    ones_t = wpool.tile([128, 512], bf16, tag="ones_a")
                    ones = ones_t[0:1, :]

                def qk_bias_mm(ps, bi, h):
                    nc.tensor.matmul(
                        ps,
                        lhsT=bias[:, bi, h * HD : (h + 1) * HD],
                        rhs=ones,
                        start=False,
                        stop=True,
                    )

                def pair_dma(pair, xt_h, xt8):
                    if pair == 0:
                        # JIT startup: the fp8 Wq + X chunks are tiny and
                        # land first, unlocking the DR waves ~1.5us in; then
                        # bf16 wq 2-ko chunks interleave with X chunks 4..15.
                        # The V-only X chunks 0..3 and phase-B constants last.
                        nc.scalar.dma_start(wq8, wqt8_p[:])
                        for k2 in range(NF8 // 2):
                            nc.sync.dma_start(
                                xt8[:, 2 * k2 : 2 * k2 + 2],
                                xt8_p[:, 0, 2 * k2 : 2 * k2 + 2],
                            )
                        for g in range(KOB // 2):
                            gko = NF8 + 2 * g
                            (nc.scalar if g % 2 else nc.sync).dma_start(
                                wq[:, 2 * g : 2 * g + 2],
                                wqt_p[:, 2 * g : 2 * g + 2],
                            )
                            (nc.sync if g % 2 else nc.scalar).dma_start(
                                xt_h[gko // 8][:, gko % 8 : gko % 8 + 2],
                                xt_p[:, 0, gko : gko + 2],
                            )
                        nc.scalar.dma_start(wk8, wkt8_p[:])
                        nc.sync.dma_start(wk, wkt_p[:])
                        nc.sync.dma_start(xt_h[0][:, 0:NF8], xt_p[:, 0, 0:NF8])
                        nc.sync.dma_start(wv, wvt_p[:])
                        nc.sync.dma_start(masks, mask_p[:])
                        nc.sync.dma_start(wot, wot_p[:])
                        nc.sync.dma_start(iden, iden_p[:])
                        if with_bias:
                            nc.sync.dma_start(bias, bias_p[:])
                            nc.sync.dma_start(ones_t, ones_p[:])
                    else:
                        nc.sync.dma_start(xt8, xt8_p[:, pair])
                        for half in range(2):
                            nc.sync.dma_start(
                                xt_h[half],
                                xt_p[:, pair, half * 8 : half * 8 + 8],
                            )

                def pair_ops(pair, xt_h, xt8):
                    """Projection compute for one 1024-token pair, as lists
                    of (cost_ns, fn) ops keyed by ('q'|'k'|'v', sub)."""
                    b = pair // 2
                    state = {}

                    def xt_at(ko, sub):
                        return xt_h[ko // 8][
                            :, ko % 8, sub * 512 : (sub + 1) * 512
                        ]

                    def qk_ops(sub, s0, w8, wt, dst, bi):
                        ops = []
                        for h in range(HPC):
                            for k2 in range(NF8 // 2):
                                def fn(sub=sub, w8=w8, bi=bi, h=h, k2=k2):
                                    key = (sub, bi, h)
                                    if k2 == 0:
                                        state[key] = psQK.tile(
                                            [128, 512], f32,
                                            tag="qk", name="psqk",
                                        )
                                    nc.tensor.matmul(
                                        state[key],
                                        lhsT=w8[
                                            :, 2 * k2 : 2 * k2 + 2,
                                            h * HD : (h + 1) * HD,
                                        ],
                                        rhs=xt8[
                                            :, 2 * k2 : 2 * k2 + 2,
                                            sub * 512 : (sub + 1) * 512,
                                        ],
                                        start=(k2 == 0),
                                        stop=False,
                                        perf_mode=DR,
                                    )
                                ops.append((231, fn))
                            for ko in range(KOB):
                                def fn(
                                    sub=sub, s0=s0, wt=wt, dst=dst,
                                    bi=bi, h=h, ko=ko,
                                ):
                                    ps = state[(sub, bi, h)]
                                    nc.tensor.matmul(
                                        ps,
                                        lhsT=wt[:, ko, h * HD : (h + 1) * HD],
                                        rhs=xt_at(ko + NF8, sub),
                                        start=False,
                                        stop=(ko == KOB - 1)
                                        and not with_bias,
                                    )
                                    if ko == KOB - 1:
                                        if with_bias:
                                            qk_bias_mm(ps, bi, h)
                                        nc.vector.tensor_copy(
                                            dst[:, b, h, s0 : s0 + 512], ps
                                        )
                                ops.append((213, fn))
                        return ops

                    res = {}
                    for sub in range(2):
                        s0 = (pair * 1024 + sub * 512) % S
                        res[("q", sub)] = qk_ops(sub, s0, wq8, wq, qt_res, 0)
                        res[("k", sub)] = qk_ops(sub, s0, wk8, wk, kt_res, 1)
                        vops = []
                        for tsub in range(4):
                            for ko in range(KO):
                                def fn(sub=sub, s0=s0, tsub=tsub, ko=ko):
                                    key = ("v", sub, tsub)
                                    if ko == 0:
                                        state[key] = psV.tile(
                                            [128, M], f32, tag="v", name="psv"
                                        )
                                    ps = state[key]
                                    nc.tensor.matmul(
                                        ps,
                                        lhsT=xt_at(ko, sub)[
                                            :, tsub * 128 : (tsub + 1) * 128
                                        ],
                                        rhs=wv[:, ko],
                                        start=(ko == 0),
                                        stop=(ko == KO - 1) and not with_bias,
                                    )
                                    if ko == KO - 1:
                                        if with_bias:
                                            nc.tensor.matmul(
                                                ps,
                                                lhsT=ones[:, :128],
                                                rhs=bias[:, 2],
                                                start=False,
                                                stop=True,
                                            )
                                        sc = (s0 + tsub * 128) // 128
                                        nc.vector.tensor_copy(
                                            v_res[:, b, :, sc, 0:HD],
                                            ps.rearrange(
                                                "p (h d) -> p h d", h=HPC
                                            ),
                                        )
                                vops.append((107, fn))
                        res[("v", sub)] = vops
                    return res

                def flat_ops(res):
                    out = []
                    for sub in range(2):
                        for kind in ("q", "k", "v"):
                            out += res[(kind, sub)]
                    return out

                def new_x_tiles():
                    xt_h = [
                        xpool.tile(
                            [128, KO // 2, 1024], bf16, tag="xt", name="xth"
                        )
                        for _ in range(2)
                    ]
                    xt8 = x8pool.tile([128, NF8, 1024], f8, tag="x8", name="x8")
                    return xt_h, xt8

                # pair 0: interleave the 4 Q chains (h, sub) wave by wave so
                # they start as soon as the first fp8/bf16 chunks land
                xt_h0, xt8_0 = new_x_tiles()
                pair_dma(0, xt_h0, xt8_0)
                ops0 = pair_ops(0, xt_h0, xt8_0)
                per = NF8 // 2 + KOB        # ops per (h, sub) Q chain
                for w in range(per):
                    for sub in range(2):
                        qsub = ops0[("q", sub)]
                        for h in range(HPC):
                            qsub[h * per + w][1]()
                # pair 0 K + V
                for key in (("k", 0), ("v", 0), ("k", 1), ("v", 1)):
                    for _, fn in ops0[key]:
                        fn()

                # pair 1: emitted bare
                xt_h1, xt8_1 = new_x_tiles()
                pair_dma(1, xt_h1, xt8_1)
                for _, fn in flat_ops(pair_ops(1, xt_h1, xt8_1)):
                    fn()

                # batch 0's Q/K/V are ready: weave groups 0+1's scores into
                # batch 1's projection compute (exps run during phase A)
                queue_scores(queue, psQK, "qk", 0, es_by)
                queue_scores(queue, psQK, "qk", 1, es_by)
                for pair in (2, 3):
                    xt_h, xt8 = new_x_tiles()
                    pair_dma(pair, xt_h, xt8)
                    # pace at >= the ~640ns exp service time: at 560 the ACT
                    # backlog fills the shared psQK ring and stalls the PE
                    acc = weave(
                        flat_ops(pair_ops(pair, xt_h, xt8)), queue, acc,
                        spacing=660.0,
                    )

            # ------------- Phase B + C: attention + out projection -------------
            with (
                tc.tile_pool(name="ctxn", bufs=12) as ctxn,
                tc.tile_pool(name="recp", bufs=12) as recp,
                tc.tile_pool(name="ctxT", bufs=2) as ctxTp,
                tc.tile_pool(name="ob", bufs=3) as obp,
            ):
                # attention-phase PSUM pools live in their own scope so the
                # bare final epilogue can reuse the banks as one deep ring
                ps_stack = ExitStack()
                psS = ps_stack.enter_context(
                    tc.tile_pool(name="psS", bufs=2, space="PSUM")
                )
                psC = ps_stack.enter_context(
                    tc.tile_pool(name="psC", bufs=3, space="PSUM")
                )
                psT = ps_stack.enter_context(
                    tc.tile_pool(name="psT", bufs=1, space="PSUM")
                )
                psO = ps_stack.enter_context(
                    tc.tile_pool(name="psO", bufs=2, space="PSUM")
                )
                def av_ops(b, qb, es, cns_out, tc_i=None):
                    """One op per k-chunk j of each 128-query chunk i; the
                    closing op of each i-chunk adds the DVE rec+normalize
                    (plus, for the last group, its transposes via tc_i)."""
                    ops = []
                    state = {}
                    for i in range(4):
                        qi = 4 * qb + i
                        for j in range(qi + 1):
                            def fn(i=i, j=j, qi=qi):
                                if j == 0:
                                    state[i] = [
                                        psC.tile(
                                            [128, 512], f32, tag="c", name="psc"
                                        )
                                        for _ in range(HPC)
                                    ]
                                pscs = state[i]
                                for h in range(HPC):
                                    nc.tensor.matmul(
                                        pscs[h][:, 0:HD1],
                                        lhsT=es[2 * j + h][
                                            :, i * 128 : (i + 1) * 128
                                        ],
                                        rhs=v_res[:, b, h, j, :],
                                        start=(j == 0),
                                        stop=(j == qi),
                                    )
                                if j == qi:
                                    cns_pair = []
                                    for h in range(HPC):
                                        rec = recp.tile(
                                            [128, 1], f32, tag="r", name="rec"
                                        )
                                        nc.vector.reciprocal(
                                            rec, pscs[h][:, HD:HD1]
                                        )
                                        cn = ctxn.tile(
                                            [128, 128], bf16, tag="cn", name="cn"
                                        )
                                        nc.vector.tensor_scalar_mul(
                                            cn, pscs[h][:, 0:HD], rec
                                        )
                                        cns_pair.append(cn)
                                    cns_out.extend(cns_pair)
                                    if tc_i is not None:
                                        tc_i(i, cns_pair)
                            ops.append((110, fn))
                    return ops

                def make_tc_t(ct):
                    """Per-chunk transposes for the final group, so the
                    epilogue is only the out projection."""
                    def tc_i(i, cns_pair):
                        for h in range(HPC):
                            pst = psT.tile([128, 512], bf16, tag="t", name="pst")
                            nc.tensor.transpose(pst[:, 0:128], cns_pair[h], iden)
                            nc.vector.tensor_copy(
                                ct[:, h, i * 128 : (i + 1) * 128], pst[:, 0:128]
                            )
                    return tc_i

                def tc_ops(b, qb, cns, ct, skip_T=False, alt_q=False,
                           cp=None, pool_o=None):
                    """Transpose normalized ctx, then the out projection.
                    Output tiles are paired into one DMA per 256 rows."""
                    ops = []
                    if not skip_T:
                        for i in range(4):
                            for h in range(HPC):
                                def fn(i=i, h=h):
                                    pst = psT.tile(
                                        [128, 512], bf16, tag="t", name="pst"
                                    )
                                    nc.tensor.transpose(
                                        pst[:, 0:128], cns[2 * i + h], iden
                                    )
                                    nc.vector.tensor_copy(
                                        ct[:, h, i * 128 : (i + 1) * 128],
                                        pst[:, 0:128],
                                    )
                                ops.append((110, fn))
                    state = {}
                    for oc in range(KO):
                        def fn(oc=oc):
                            pso = (pool_o or psO).tile(
                                [128, 512], f32, tag="o", name="pso"
                            )
                            for h in range(HPC):
                                nc.tensor.matmul(
                                    pso,
                                    lhsT=wot[:, h, oc],
                                    rhs=ct[:, h, :],
                                    start=(h == 0),
                                    stop=(h == HPC - 1),
                                )
                            def ccopy(dst, src, oc=oc):
                                eng = cp[oc % len(cp)] if cp else "dve"
                                if eng == "split":
                                    # halves on both engines concurrently:
                                    # frees the PSUM bank in ~400ns so the
                                    # epilogue stays PE-bound
                                    nc.scalar.activation(
                                        dst[:, 0:256], src[:, 0:256], AF.Copy
                                    )
                                    nc.vector.tensor_copy(
                                        dst[:, 256:512], src[:, 256:512]
                                    )
                                elif eng == "act":
                                    nc.scalar.activation(dst, src, AF.Copy)
                                else:
                                    nc.vector.tensor_copy(dst, src)
                            if oc % 2 == 0:
                                state["ob"] = obp.tile(
                                    [128, 2, 512], f16, tag="ob", name="ob"
                                )
                                ccopy(state["ob"][:, 0], pso)
                            else:
                                ob = state["ob"]
                                ccopy(ob[:, 1], pso)
                                eng = (
                                    nc.scalar
                                    if alt_q and (oc // 2) % 2
                                    else nc.sync
                                )
                                eng.dma_start(
                                    out_p[
                                        b,
                                        (oc - 1) * 128 : (oc + 1) * 128,
                                        qb * 512 : (qb + 1) * 512,
                                    ].rearrange("(u p) s -> p u s", u=2),
                                    ob,
                                )
                        ops.append((430, fn))
                    return ops

                prev = None
                last_tc = None
                for gi, (b, qb) in enumerate(groups):
                    if gi + 2 < len(groups):
                        queue_scores(queue, psS, "s", gi + 2, es_by)
                    last = gi == len(groups) - 1
                    # late groups: exps are mostly done so ACT has slack,
                    # while DVE carries the AV normalize chain -- stage the
                    # out tiles on ACT to keep DVE off the critical path.
                    # (Measured dead ends: split-halves for gi>=4 or
                    # act/dve+split mixes for gi>=5 both regress 7-15us.)
                    cp = ["act"] if gi >= 5 else None
                    pre_ops = [] if prev is None else tc_ops(*prev, cp=cp)
                    cns = []
                    ct = ctxTp.tile([128, HPC, 512], bf16, tag="ct", name="ct")
                    if not last:
                        acc = weave(pre_ops, queue, acc)
                        # barrier: scores(g) all emitted before AV(g)
                        while queue and queue[0][0] <= gi:
                            queue.popleft()[1]()
                            acc = 0.0
                        acc = weave(
                            av_ops(b, qb, es_by[gi], cns, tc_i=None),
                            queue, acc,
                        )
                        prev = (b, qb, cns, ct)
                    else:
                        # final group: drain the queue inside the first part
                        # of prev's out-projection, then interleave the tiny
                        # AV so its DVE-latency chain hides under PE work
                        # and the epilogue can start immediately after
                        n_head = min(12, len(pre_ops))
                        acc = weave(pre_ops[:n_head], queue, acc)
                        while queue and queue[0][0] <= gi:
                            queue.popleft()[1]()
                            acc = 0.0
                        avl = av_ops(
                            b, qb, es_by[gi], cns, tc_i=make_tc_t(ct)
                        )
                        rest = pre_ops[n_head:]
                        merged = []
                        ai = 0
                        for k, op in enumerate(rest):
                            merged.append(op)
                            want = (k + 1) * len(avl) // max(len(rest), 1)
                            while ai < min(want, len(avl)):
                                merged.append(avl[ai])
                                ai += 1
                        merged.extend(avl[ai:])
                        for _, fn in merged:
                            fn()
                        last_tc = (b, qb, cns, ct)
                        prev = None
                    del es_by[gi]
                if prev is not None:
                    for _, fn in tc_ops(*prev):
                        fn()
                if last_tc is not None:
                    ps_stack.close()
                    with tc.tile_pool(
                        name="psE", bufs=6, space="PSUM"
                    ) as psE:
                        for _, fn in tc_ops(
                            *last_tc, skip_T=True, alt_q=True,
                            cp=["act", "dve"], pool_o=psE,
                        ):
                            fn()
                else:
                    ps_stack.close()

    nc.finalize()
    return nc


def _get_nc(with_bias=False):
    if with_bias not in _built:
        _built[with_bias] = _build(with_bias)
    return _built[with_bias]


def kernel(hidden_states, attention_mask, Wq, bq, Wk, bk, Wv, bv, Wo, bo):
    hidden_states = np.asarray(hidden_states, dtype=np.float32)
    Wq, Wk, Wv, Wo = (np.asarray(w, dtype=np.float32) for w in (Wq, Wk, Wv, Wo))
    bq, bk, bv, bo = (np.asarray(v, dtype=np.float32) for v in (bq, bk, bv, bo))

    with_bias = bool(np.any(bq) or np.any(bk) or np.any(bv))

    x8 = hidden_states.reshape(T, D) * 8.0
    # XT[p, pair, ko, t] = 8 * x[pair*1024 + t, ko*128 + p]
    xt = np.ascontiguousarray(
        x8.reshape(NPAIR, 1024, KO, 128).transpose(3, 0, 2, 1)
    ).astype(BF16)
    # fp8 copy of chunks 0..NF8-1 at the same 8x scale (Q/K DoubleRow part)
    xt8 = np.ascontiguousarray(
        np.clip(x8[:, : NF8 * 128], -F8MAX, F8MAX)
        .reshape(NPAIR, 1024, NF8, 128)
        .transpose(3, 0, 2, 1)
    ).astype(F8)

    # causal 0/1 mask for the single [128,128] chunk on the diagonal:
    # mask[p, f] = p <= f
    mask = (np.arange(128)[:, None] <= np.arange(128)[None, :]).astype(BF16)
    iden = np.eye(128, dtype=BF16)
    ones = np.ones((128, 512), dtype=BF16)

    in_maps = []
    d8 = NF8 * 128
    for c in range(NCORES):
        rows = slice(c * M, (c + 1) * M)
        # bf16 W*T chunks NF8.. at 512x scale (matches the fp8 part's scale)
        wqt = np.ascontiguousarray(
            (Wq[rows, d8:].T * 512.0).reshape(KOB, 128, M).transpose(1, 0, 2)
        ).astype(BF16)
        wkt = np.ascontiguousarray(
            (Wk[rows, d8:].T * 512.0).reshape(KOB, 128, M).transpose(1, 0, 2)
        ).astype(BF16)
        wqt8 = np.ascontiguousarray(
            np.clip(Wq[rows, :d8].T * 512.0, -F8MAX, F8MAX)
            .reshape(NF8, 128, M)
            .transpose(1, 0, 2)
        ).astype(F8)
        wkt8 = np.ascontiguousarray(
            np.clip(Wk[rows, :d8].T * 512.0, -F8MAX, F8MAX)
            .reshape(NF8, 128, M)
            .transpose(1, 0, 2)
        ).astype(F8)
        # Wv/8 cancels X's 8x so V lands at natural scale
        wvt = np.ascontiguousarray(
            (Wv[rows, :].T / 8.0).reshape(KO, 128, M).transpose(1, 0, 2)
        ).astype(BF16)
        # WOT[p, h, oc, j] = Wo[oc*128 + j, rows0 + h*128 + p]
        wot = np.ascontiguousarray(
            Wo[:, rows].reshape(KO, 128, HPC, 128).transpose(3, 2, 0, 1)
        ).astype(BF16)
        bias = np.stack(
            [bq[rows] * 4096.0, bk[rows] * 4096.0, bv[rows]]
        )[None].astype(BF16)
        in_maps.append(
            {
                "XT": xt,
                "XT8": xt8,
                "WQT": wqt,
                "WKT": wkt,
                "WQT8": wqt8,
                "WKT8": wkt8,
                "WVT": wvt,
                "WOT": wot,
                "BIAS": np.ascontiguousarray(bias),
                "MASK": mask,
                "IDEN": iden,
                "ONES": ones,
            }
        )

    res = run_bass_kernel_spmd(_get_nc(with_bias), in_maps, list(range(NCORES)))
    out = res.results[0]["OUT"].astype(np.float32)
    for c in range(1, NCORES):
        out += res.results[c]["OUT"].astype(np.float32)
    out = np.ascontiguousarray(out.transpose(0, 2, 1))
    out += bo
    return out



# revision 55
# speedup vs baseline: 1.1772x; 1.0042x over previous
"""Causal multi-head attention on 8 trn2 NeuronCores.

Problem: B=2, S=2048, D=2048, H=16 (HD=128), fp32 in/out.
Sharding: tensor-parallel over heads - core c owns heads {2c, 2c+1} for both
batches. Each core computes its Q/K/V projections, attention for its 4
(batch, head) pairs, and a partial output projection over its head slice.
The host sums the 8 partial outputs (transposing [B,D,S] -> [B,S,D]) and
adds the output bias.

Operands are bf16 in SBUF (fp32 PSUM accumulation) except the first NF8=6
of 16 contraction chunks of the Q/K projections, which run as fp8e4
DoubleRow matmuls (2 chunks per instruction at 2x PE rate). The fp8 logit
noise costs ~1.6e-2 max-norm rel err (gate 2e-2); the numpy sim in the
transcript tracked hardware within 1%, and any fp8 on the V/ctx/Wo path
blows the budget (heavy-tailed ctx), so everything else stays bf16.

Device algorithm (per core):
  Phase A: stream X^T (bf16 at 8x scale, plus an fp8 copy of the first NF8
           chunks) in 2KB-line DMAs; compute Q^T/K^T (head-dim on
           partitions, 512x-scaled weights, fp8-DR head + bf16 tail) and V
           (tokens on partitions, Wv/8 so V is natural scale); all
           SBUF-resident. V gets a ones-column appended ([V | 1]).
  Phase B: per (b, qb of 512 queries): score tiles S^T = K^T_chunk.T @ Q^T
           (k on partitions; the 4096^2 scale folds into the exp scale).
           Diagonal-block tiles only compute/exp the valid column range
           [t_loc*128, 512) and Pool-mask the single [128,128] diagonal
           chunk. Then per 128-query chunk i: ctx_ext[q, 0:129] =
           sum_j E_chunk(j).T @ [V|1] accumulated in PSUM - column 128 is
           the softmax denominator for free. A [128,1] DVE reciprocal + DVE
           per-partition-scalar multiply normalizes ctx into bf16, and a PE
           transpose flips it to [hd, q].
  Phase C: out^T tiles = sum_h Wo_chunk.T @ ctx^T, staged to fp16 and
           written as OUT[b, D, S]; the host sums the 8 fp16 partials in
           fp32 (fp16 OUT halves the HBM write traffic; fp32 OUT saturated
           the DMA queues at the tail).

  ACT exp (~640ns/tile) is slower than the 213ns score matmul, so score ops
  are WOVEN into the rest of the PE stream at one per ~560ns of PE time
  (660ns during phase A, where exps share the psQK ring with the
  projection chains and a faster pace fills it) with a 2-group lookahead;
  the first two groups' scores hide inside phase A's batch-1 projections.
  Batch 1's groups run qb-descending so the run ends on the smallest
  group; its AV interleaves into the previous group's out-projection, and
  the bare final epilogue closes the attention PSUM pools and reruns from
  a fresh 6-bank ring with copies split across ACT+DVE halves, so the
  ~610ns bank-free latency pipelines behind the 462ns matmul pairs.
  No max-subtraction is needed: scores are O(8) for this problem so exp
  cannot overflow, and softmax is shift-invariant.
"""

import os
from collections import deque
from contextlib import ExitStack

import numpy as np
import ml_dtypes

import concourse.bacc as bacc
import concourse.tile as tile
from concourse import mybir
from concourse.bass_utils import run_bass_kernel_spmd

BF16 = ml_dtypes.bfloat16
F8 = ml_dtypes.float8_e4m3
F8MAX = 240.0


def _install_neff_cache():
    """Cache compiled NEFFs on disk keyed by BIR content hash.

    Purely a compile-time memo: identical BIR -> identical NEFF, so repeat
    runs skip the multi-minute neuronxcc compile. No effect on execution.
    """
    import hashlib
    import shutil

    import concourse.bass2jax as _b2j
    import concourse.bass_utils as _bu

    if getattr(_bu, "_neff_cache_installed", False):
        return
    cache_dir = os.environ.get("NEFF_CACHE_DIR", "/tmp/neff_cache")
    orig = _bu.compile_bir_kernel

    def cached(bir_json, tmpdir, neff_name="file.neff"):
        try:
            os.makedirs(cache_dir, exist_ok=True)
            key = hashlib.sha256(bir_json).hexdigest()[:24]
            cpath = os.path.join(cache_dir, key + ".neff")
            dst = os.path.join(tmpdir, neff_name)
            if os.path.exists(cpath):
                shutil.copy(cpath, dst)
                return dst
            out = orig(bir_json, tmpdir, neff_name)
            shutil.copy(out, cpath)
            return out
        except OSError:
            return orig(bir_json, tmpdir, neff_name)

    _bu.compile_bir_kernel = cached
    _b2j.compile_bir_kernel = cached
    _bu._neff_cache_installed = True


_install_neff_cache()

B, S, D, H = 2, 2048, 2048, 16
HD = D // H          # 128
NCORES = 8
HPC = H // NCORES    # heads per core = 2
M = HPC * HD         # 256 output columns per core per projection
T = B * S            # 4096 total token rows
KO = D // 128        # 16 contraction chunks
NPAIR = T // 1024    # 4 phase-A token pairs of 1024
QB = S // 512        # 4 query blocks per batch
SC = S // 128        # 16 key chunks per sequence
HD1 = HD + 1         # V with ones column
SCALE = 1.0 / float(np.sqrt(HD))
# Q/K projections: the first NF8 contraction chunks run as fp8e4 DoubleRow
# matmuls (2 chunks per instruction at 2x rate). Host pre-scales X by 8 and
# Wq/Wk by 512 (exact powers of 2) so the fp8 and bf16 partial products
# accumulate at one consistent 4096x scale; exp() divides it back out.
# NF8=6 keeps the extra logit noise at ~1.6e-2 rel err (budget 2e-2);
# hardware matched the numpy fp8 sim within 1% at NF8=4 (1.281 vs 1.293e-2).
NF8 = 6
KOB = KO - NF8       # bf16 contraction chunks of Q/K
QKSC = 8.0 * 512.0
ESCALE = SCALE / (QKSC * QKSC)
# V projection: first NF8V chunks fp8-DR, reusing the same fp8 X tiles.
# Wv carries 64x (bf16 part too); the PSUM->SBUF copy divides by 8*64=512.
# Sim: nf8v=2 leaves rel err at 1.60e-2 (nf8v=4 would hit 1.99e-2).
NF8V = 2
KOBV = KO - NF8V     # bf16 contraction chunks of V

_built = {}


def _build(with_bias):
    f32 = mybir.dt.float32
    f16 = mybir.dt.float16
    bf16 = mybir.dt.bfloat16
    AF = mybir.ActivationFunctionType

    f8 = mybir.dt.float8e4
    DR = mybir.MatmulPerfMode.DoubleRow

    nc = bacc.Bacc(None, target_bir_lowering=False)

    # ---- per-core DRAM parameters (host supplies per-core shards) ----
    # XT[p, pair, ko, t] = 8 * x[pair*1024 + t, ko*128 + p]
    xt_p = nc.declare_dram_parameter("XT", [128, NPAIR, KO, 1024], bf16, False)
    # fp8 copy of the first NF8 chunks (same 8x scale) for the Q/K DR part
    xt8_p = nc.declare_dram_parameter(
        "XT8", [128, NPAIR, NF8, 1024], f8, False
    )
    # WqT/WkT[p, ko, m] = 512 * W[rows0 + m, (ko+NF8)*128 + p]  (bf16 part)
    wqt_p = nc.declare_dram_parameter("WQT", [128, KOB, M], bf16, False)
    wkt_p = nc.declare_dram_parameter("WKT", [128, KOB, M], bf16, False)
    # fp8 Wq/Wk chunks 0..NF8-1 at the same 512x scale
    wqt8_p = nc.declare_dram_parameter("WQT8", [128, NF8, M], f8, False)
    wkt8_p = nc.declare_dram_parameter("WKT8", [128, NF8, M], f8, False)
    # WvT at natural/8 scale so V comes out at natural scale (X carries 8x)
    wvt_p = nc.declare_dram_parameter("WVT", [128, KO, M], bf16, False)
    # WOT[p, h, oc, j] = Wo[oc*128 + j, rows0 + h*128 + p]
    wot_p = nc.declare_dram_parameter("WOT", [128, HPC, KO, 128], bf16, False)
    bias_p = nc.declare_dram_parameter("BIAS", [1, 3, M], bf16, False)
    mask_p = nc.declare_dram_parameter("MASK", [128, 128], bf16, False)
    iden_p = nc.declare_dram_parameter("IDEN", [128, 128], bf16, False)
    ones_p = nc.declare_dram_parameter("ONES", [128, 512], bf16, False)
    # fp16 partial outputs: halves HBM write traffic vs fp32, and the final
    # group's OUT burst no longer saturates the DMA queues at the tail
    out_p = nc.declare_dram_parameter("OUT", [B, D, S], f16, True)

    # batch 1 runs qb descending so the run ends on the smallest group
    # (1, 0): 8 exps and a 10-op AV keep the tail dependency chain short
    groups = [(0, 0), (0, 1), (0, 2), (0, 3), (1, 3), (1, 2), (1, 1), (1, 0)]
    SPACING = 560.0

    with tile.TileContext(nc) as tc:
        with (
            tc.tile_pool(name="persist", bufs=1) as persist,
            tc.tile_pool(name="bconst", bufs=1) as bconst,
            tc.tile_pool(name="epool", bufs=64) as epool,
        ):
            qt_res = persist.tile([128, B, HPC, S], bf16)
            kt_res = persist.tile([128, B, HPC, S], bf16)
            v_res = persist.tile([128, B, HPC, SC, HD1], bf16)
            # ones column of [V | 1]; disjoint from the phase-A V writes
            nc.vector.memset(v_res[:, :, :, :, HD:HD1], 1.0)

            # phase-B/C constants (DMAs queued below, after pair-0's X)
            masks = bconst.tile([128, 128], bf16, tag="masks")
            wot = bconst.tile([128, HPC, KO, 128], bf16, tag="wot")
            iden = bconst.tile([128, 128], bf16, tag="iden")

            def make_score_op(pool, tag, b, qb, t, h, out_list):
                def fn():
                    pss = pool.tile([128, 512], f32, tag=tag, name="pss")
                    e = epool.tile([128, 512], bf16, tag="e", name="e")
                    lhsT = kt_res[:, b, h, t * 128 : (t + 1) * 128]
                    t_loc = t - 4 * qb
                    if t_loc < 0:
                        # fully below the diagonal: whole tile is valid
                        nc.tensor.matmul(
                            pss,
                            lhsT=lhsT,
                            rhs=qt_res[:, b, h, qb * 512 : (qb + 1) * 512],
                            start=True,
                            stop=True,
                        )
                        nc.scalar.activation(e, pss, AF.Exp, scale=ESCALE)
                    else:
                        # diagonal-block tile: queries < t*128 are masked, so
                        # only compute cols [t_loc*128, 512). AV(i, j) reads
                        # es[j] col-chunk i only for i >= t_loc, so the
                        # unwritten low columns are never consumed.
                        c0 = t_loc * 128
                        nc.tensor.matmul(
                            pss[:, c0:512],
                            lhsT=lhsT,
                            rhs=qt_res[
                                :, b, h, qb * 512 + c0 : (qb + 1) * 512
                            ],
                            start=True,
                            stop=True,
                        )
                        nc.scalar.activation(
                            e[:, c0:512], pss[:, c0:512], AF.Exp, scale=ESCALE
                        )
                        # only the [128,128] chunk ON the diagonal needs the
                        # triangular mask; it runs on the idle Pool engine
                        nc.gpsimd.tensor_mul(
                            e[:, c0 : c0 + 128], e[:, c0 : c0 + 128], masks
                        )
                    out_list.append(e)

                return fn

            def weave(pe_ops, queue, acc, spacing=SPACING):
                """Emit pe_ops, inserting one queued score per `spacing` ns
                of accumulated PE time. Returns the leftover accum."""
                for cost, fn in pe_ops:
                    while queue and acc >= spacing:
                        queue.popleft()[1]()
                        acc -= spacing
                    fn()
                    acc += cost
                return acc

            def queue_scores(queue, pool, tag, gi, es_by):
                es_by[gi] = []
                b, qb = groups[gi]
                for t in range(4 * (qb + 1)):
                    for h in range(HPC):
                        queue.append(
                            (gi, make_score_op(pool, tag, b, qb, t, h, es_by[gi]))
                        )

            es_by = {}
            queue = deque()
            acc = 0.0

            # ---------------- Phase A: projections ----------------
            with (
                tc.tile_pool(name="wqkv", bufs=1) as wpool,
                tc.tile_pool(name="xs", bufs=3) as xpool,
                tc.tile_pool(name="x8s", bufs=3) as x8pool,
                tc.tile_pool(name="psQK", bufs=6, space="PSUM") as psQK,
                tc.tile_pool(name="psV", bufs=2, space="PSUM") as psV,
            ):
                wq = wpool.tile([128, KOB, M], bf16, tag="wq")
                wk = wpool.tile([128, KOB, M], bf16, tag="wk")
                wq8 = wpool.tile([128, NF8, M], f8, tag="wq8")
                wk8 = wpool.tile([128, NF8, M], f8, tag="wk8")
                wv = wpool.tile([128, KO, M], bf16, tag="wv")
                if with_bias:
                    bias = wpool.tile([1, 3, M], bf16, tag="bias")
                    ones_t = wpool.tile([128, 512], bf16, tag="ones_a")
                    ones = ones_t[0:1, :]

                def qk_bias_mm(ps, bi, h):
                    nc.tensor.matmul(
                        ps,
                        lhsT=bias[:, bi, h * HD : (h + 1) * HD],
                        rhs=ones,
                        start=False,
                        stop=True,
                    )

                def pair_dma(pair, xt_h, xt8):
                    if pair == 0:
                        # JIT startup: the fp8 Wq + X chunks are tiny and
                        # land first, unlocking the DR waves ~1.5us in; then
                        # bf16 wq 2-ko chunks interleave with X chunks 4..15.
                        # The V-only X chunks 0..3 and phase-B constants last.
                        nc.scalar.dma_start(wq8, wqt8_p[:])
                        for k2 in range(NF8 // 2):
                            nc.sync.dma_start(
                                xt8[:, 2 * k2 : 2 * k2 + 2],
                                xt8_p[:, 0, 2 * k2 : 2 * k2 + 2],
                            )
                        for g in range(KOB // 2):
                            gko = NF8 + 2 * g
                            (nc.scalar if g % 2 else nc.sync).dma_start(
                                wq[:, 2 * g : 2 * g + 2],
                                wqt_p[:, 2 * g : 2 * g + 2],
                            )
                            (nc.sync if g % 2 else nc.scalar).dma_start(
                                xt_h[gko // 8][:, gko % 8 : gko % 8 + 2],
                                xt_p[:, 0, gko : gko + 2],
                            )
                        nc.scalar.dma_start(wk8, wkt8_p[:])
                        nc.sync.dma_start(wk, wkt_p[:])
                        nc.sync.dma_start(xt_h[0][:, 0:NF8], xt_p[:, 0, 0:NF8])
                        nc.sync.dma_start(wv, wvt_p[:])
                        nc.sync.dma_start(masks, mask_p[:])
                        nc.sync.dma_start(wot, wot_p[:])
                        nc.sync.dma_start(iden, iden_p[:])
                        if with_bias:
                            nc.sync.dma_start(bias, bias_p[:])
                            nc.sync.dma_start(ones_t, ones_p[:])
                    else:
                        nc.sync.dma_start(xt8, xt8_p[:, pair])
                        for half in range(2):
                            nc.sync.dma_start(
                                xt_h[half],
                                xt_p[:, pair, half * 8 : half * 8 + 8],
                            )

                def pair_ops(pair, xt_h, xt8):
                    """Projection compute for one 1024-token pair, as lists
                    of (cost_ns, fn) ops keyed by ('q'|'k'|'v', sub)."""
                    b = pair // 2
                    state = {}

                    def xt_at(ko, sub):
                        return xt_h[ko // 8][
                            :, ko % 8, sub * 512 : (sub + 1) * 512
                        ]

                    def qk_ops(sub, s0, w8, wt, dst, bi):
                        ops = []
                        for h in range(HPC):
                            for k2 in range(NF8 // 2):
                                def fn(sub=sub, w8=w8, bi=bi, h=h, k2=k2):
                                    key = (sub, bi, h)
                                    if k2 == 0:
                                        state[key] = psQK.tile(
                                            [128, 512], f32,
                                            tag="qk", name="psqk",
                                        )
                                    nc.tensor.matmul(
                                        state[key],
                                        lhsT=w8[
                                            :, 2 * k2 : 2 * k2 + 2,
                                            h * HD : (h + 1) * HD,
                                        ],
                                        rhs=xt8[
                                            :, 2 * k2 : 2 * k2 + 2,
                                            sub * 512 : (sub + 1) * 512,
                                        ],
                                        start=(k2 == 0),
                                        stop=False,
                                        perf_mode=DR,
                                    )
                                ops.append((231, fn))
                            for ko in range(KOB):
                                def fn(
                                    sub=sub, s0=s0, wt=wt, dst=dst,
                                    bi=bi, h=h, ko=ko,
                                ):
                                    ps = state[(sub, bi, h)]
                                    nc.tensor.matmul(
                                        ps,
                                        lhsT=wt[:, ko, h * HD : (h + 1) * HD],
                                        rhs=xt_at(ko + NF8, sub),
                                        start=False,
                                        stop=(ko == KOB - 1)
                                        and not with_bias,
                                    )
                                    if ko == KOB - 1:
                                        if with_bias:
                                            qk_bias_mm(ps, bi, h)
                                        nc.vector.tensor_copy(
                                            dst[:, b, h, s0 : s0 + 512], ps
                                        )
                                ops.append((213, fn))
                        return ops

                    res = {}
                    for sub in range(2):
                        s0 = (pair * 1024 + sub * 512) % S
                        res[("q", sub)] = qk_ops(sub, s0, wq8, wq, qt_res, 0)
                        res[("k", sub)] = qk_ops(sub, s0, wk8, wk, kt_res, 1)
                        vops = []
                        for tsub in range(4):
                            for ko in range(KO):
                                def fn(sub=sub, s0=s0, tsub=tsub, ko=ko):
                                    key = ("v", sub, tsub)
                                    if ko == 0:
                                        state[key] = psV.tile(
                                            [128, M], f32, tag="v", name="psv"
                                        )
                                    ps = state[key]
                                    nc.tensor.matmul(
                                        ps,
                                        lhsT=xt_at(ko, sub)[
                                            :, tsub * 128 : (tsub + 1) * 128
                                        ],
                                        rhs=wv[:, ko],
                                        start=(ko == 0),
                                        stop=(ko == KO - 1) and not with_bias,
                                    )
                                    if ko == KO - 1:
                                        if with_bias:
                                            nc.tensor.matmul(
                                                ps,
                                                lhsT=ones[:, :128],
                                                rhs=bias[:, 2],
                                                start=False,
                                                stop=True,
                                            )
                                        sc = (s0 + tsub * 128) // 128
                                        nc.vector.tensor_copy(
                                            v_res[:, b, :, sc, 0:HD],
                                            ps.rearrange(
                                                "p (h d) -> p h d", h=HPC
                                            ),
                                        )
                                vops.append((107, fn))
                        res[("v", sub)] = vops
                    return res

                def flat_ops(res):
                    out = []
                    for sub in range(2):
                        for kind in ("q", "k", "v"):
                            out += res[(kind, sub)]
                    return out

                def new_x_tiles():
                    xt_h = [
                        xpool.tile(
                            [128, KO // 2, 1024], bf16, tag="xt", name="xth"
                        )
                        for _ in range(2)
                    ]
                    xt8 = x8pool.tile([128, NF8, 1024], f8, tag="x8", name="x8")
                    return xt_h, xt8

                # pair 0: interleave the 4 Q chains (h, sub) wave by wave so
                # they start as soon as the first fp8/bf16 chunks land
                xt_h0, xt8_0 = new_x_tiles()
                pair_dma(0, xt_h0, xt8_0)
                ops0 = pair_ops(0, xt_h0, xt8_0)
                per = NF8 // 2 + KOB        # ops per (h, sub) Q chain
                for w in range(per):
                    for sub in range(2):
                        qsub = ops0[("q", sub)]
                        for h in range(HPC):
                            qsub[h * per + w][1]()
                # pair 0 K + V
                for key in (("k", 0), ("v", 0), ("k", 1), ("v", 1)):
                    for _, fn in ops0[key]:
                        fn()

                # pair 1: emitted bare
                xt_h1, xt8_1 = new_x_tiles()
                pair_dma(1, xt_h1, xt8_1)
                for _, fn in flat_ops(pair_ops(1, xt_h1, xt8_1)):
                    fn()

                # batch 0's Q/K/V are ready: weave groups 0+1's scores into
                # batch 1's projection compute (exps run during phase A)
                queue_scores(queue, psQK, "qk", 0, es_by)
                queue_scores(queue, psQK, "qk", 1, es_by)
                for pair in (2, 3):
                    xt_h, xt8 = new_x_tiles()
                    pair_dma(pair, xt_h, xt8)
                    # pace at >= the ~640ns exp service time: at 560 the ACT
                    # backlog fills the shared psQK ring and stalls the PE
                    acc = weave(
                        flat_ops(pair_ops(pair, xt_h, xt8)), queue, acc,
                        spacing=660.0,
                    )

            # ------------- Phase B + C: attention + out projection -------------
            with (
                tc.tile_pool(name="ctxn", bufs=12) as ctxn,
                tc.tile_pool(name="recp", bufs=12) as recp,
                tc.tile_pool(name="ctxT", bufs=2) as ctxTp,
                tc.tile_pool(name="ob", bufs=3) as obp,
            ):
                # attention-phase PSUM pools live in their own scope so the
                # bare final epilogue can reuse the banks as one deep ring
                ps_stack = ExitStack()
                psS = ps_stack.enter_context(
                    tc.tile_pool(name="psS", bufs=2, space="PSUM")
                )
                psC = ps_stack.enter_context(
                    tc.tile_pool(name="psC", bufs=3, space="PSUM")
                )
                psT = ps_stack.enter_context(
                    tc.tile_pool(name="psT", bufs=1, space="PSUM")
                )
                psO = ps_stack.enter_context(
                    tc.tile_pool(name="psO", bufs=2, space="PSUM")
                )
                def av_ops(b, qb, es, cns_out, tc_i=None):
                    """One op per k-chunk j of each 128-query chunk i; the
                    closing op of each i-chunk adds the DVE rec+normalize
                    (plus, for the last group, its transposes via tc_i)."""
                    ops = []
                    state = {}
                    for i in range(4):
                        qi = 4 * qb + i
                        for j in range(qi + 1):
                            def fn(i=i, j=j, qi=qi):
                                if j == 0:
                                    state[i] = [
                                        psC.tile(
                                            [128, 512], f32, tag="c", name="psc"
                                        )
                                        for _ in range(HPC)
                                    ]
                                pscs = state[i]
                                for h in range(HPC):
                                    nc.tensor.matmul(
                                        pscs[h][:, 0:HD1],
                                        lhsT=es[2 * j + h][
                                            :, i * 128 : (i + 1) * 128
                                        ],
                                        rhs=v_res[:, b, h, j, :],
                                        start=(j == 0),
                                        stop=(j == qi),
                                    )
                                if j == qi:
                                    cns_pair = []
                                    for h in range(HPC):
                                        rec = recp.tile(
                                            [128, 1], f32, tag="r", name="rec"
                                        )
                                        nc.vector.reciprocal(
                                            rec, pscs[h][:, HD:HD1]
                                        )
                                        cn = ctxn.tile(
                                            [128, 128], bf16, tag="cn", name="cn"
                                        )
                                        nc.vector.tensor_scalar_mul(
                                            cn, pscs[h][:, 0:HD], rec
                                        )
                                        cns_pair.append(cn)
                                    cns_out.extend(cns_pair)
                                    if tc_i is not None:
                                        tc_i(i, cns_pair)
                            ops.append((110, fn))
                    return ops

                def make_tc_t(ct):
                    """Per-chunk transposes for the final group, so the
                    epilogue is only the out projection."""
                    def tc_i(i, cns_pair):
                        for h in range(HPC):
                            pst = psT.tile([128, 512], bf16, tag="t", name="pst")
                            nc.tensor.transpose(pst[:, 0:128], cns_pair[h], iden)
                            nc.vector.tensor_copy(
                                ct[:, h, i * 128 : (i + 1) * 128], pst[:, 0:128]
                            )
                    return tc_i

                def tc_ops(b, qb, cns, ct, skip_T=False, alt_q=False,
                           cp=None, pool_o=None):
                    """Transpose normalized ctx, then the out projection.
                    Output tiles are paired into one DMA per 256 rows."""
                    ops = []
                    if not skip_T:
                        for i in range(4):
                            for h in range(HPC):
                                def fn(i=i, h=h):
                                    pst = psT.tile(
                                        [128, 512], bf16, tag="t", name="pst"
                                    )
                                    nc.tensor.transpose(
                                        pst[:, 0:128], cns[2 * i + h], iden
                                    )
                                    nc.vector.tensor_copy(
                                        ct[:, h, i * 128 : (i + 1) * 128],
                                        pst[:, 0:128],
                                    )
                                ops.append((110, fn))
                    state = {}
                    for oc in range(KO):
                        def fn(oc=oc):
                            pso = (pool_o or psO).tile(
                                [128, 512], f32, tag="o", name="pso"
                            )
                            for h in range(HPC):
                                nc.tensor.matmul(
                                    pso,
                                    lhsT=wot[:, h, oc],
                                    rhs=ct[:, h, :],
                                    start=(h == 0),
                                    stop=(h == HPC - 1),
                                )
                            def ccopy(dst, src, oc=oc):
                                eng = cp[oc % len(cp)] if cp else "dve"
                                if eng == "split":
                                    # halves on both engines concurrently:
                                    # frees the PSUM bank in ~400ns so the
                                    # epilogue stays PE-bound
                                    nc.scalar.activation(
                                        dst[:, 0:256], src[:, 0:256], AF.Copy
                                    )
                                    nc.vector.tensor_copy(
                                        dst[:, 256:512], src[:, 256:512]
                                    )
                                elif eng == "act":
                                    nc.scalar.activation(dst, src, AF.Copy)
                                else:
                                    nc.vector.tensor_copy(dst, src)
                            if oc % 2 == 0:
                                state["ob"] = obp.tile(
                                    [128, 2, 512], f16, tag="ob", name="ob"
                                )
                                ccopy(state["ob"][:, 0], pso)
                            else:
                                ob = state["ob"]
                                ccopy(ob[:, 1], pso)
                                eng = (
                                    nc.scalar
                                    if alt_q and (oc // 2) % 2
                                    else nc.sync
                                )
                                eng.dma_start(
                                    out_p[
                                        b,
                                        (oc - 1) * 128 : (oc + 1) * 128,
                                        qb * 512 : (qb + 1) * 512,
                                    ].rearrange("(u p) s -> p u s", u=2),
                                    ob,
                                )
                        ops.append((430, fn))
                    return ops

                prev = None
                last_tc = None
                for gi, (b, qb) in enumerate(groups):
                    if gi + 2 < len(groups):
                        queue_scores(queue, psS, "s", gi + 2, es_by)
                    last = gi == len(groups) - 1
                    # late groups: exps are mostly done so ACT has slack,
                    # while DVE carries the AV normalize chain -- stage the
                    # out tiles on ACT to keep DVE off the critical path.
                    # (Measured dead ends: split-halves for gi>=4 or
                    # act/dve+split mixes for gi>=5 both regress 7-15us.)
                    cp = ["act"] if gi >= 5 else None
                    pre_ops = [] if prev is None else tc_ops(*prev, cp=cp)
                    cns = []
                    ct = ctxTp.tile([128, HPC, 512], bf16, tag="ct", name="ct")
                    if not last:
                        acc = weave(pre_ops, queue, acc)
                        # barrier: scores(g) all emitted before AV(g)
                        while queue and queue[0][0] <= gi:
                            queue.popleft()[1]()
                            acc = 0.0
                        acc = weave(
                            av_ops(b, qb, es_by[gi], cns, tc_i=None),
                            queue, acc,
                        )
                        prev = (b, qb, cns, ct)
                    else:
                        # final group: drain the queue inside the first part
                        # of prev's out-projection, then interleave the tiny
                        # AV so its DVE-latency chain hides under PE work
                        # and the epilogue can start immediately after
                        n_head = min(8, len(pre_ops))
                        acc = weave(pre_ops[:n_head], queue, acc)
                        while queue and queue[0][0] <= gi:
                            queue.popleft()[1]()
                            acc = 0.0
                        avl = av_ops(
                            b, qb, es_by[gi], cns, tc_i=make_tc_t(ct)
                        )
                        rest = pre_ops[n_head:]
                        merged = []
                        ai = 0
                        for k, op in enumerate(rest):
                            merged.append(op)
                            want = (k + 1) * len(avl) // max(len(rest), 1)
                            while ai < min(want, len(avl)):
                                merged.append(avl[ai])
                                ai += 1
                        merged.extend(avl[ai:])
                        for _, fn in merged:
                            fn()
                        last_tc = (b, qb, cns, ct)
                        prev = None
                    del es_by[gi]
                if prev is not None:
                    for _, fn in tc_ops(*prev):
                        fn()
                if last_tc is not None:
                    ps_stack.close()
                    with tc.tile_pool(
                        name="psE", bufs=6, space="PSUM"
                    ) as psE:
                        for _, fn in tc_ops(
                            *last_tc, skip_T=True, alt_q=True,
                            cp=["act", "dve"], pool_o=psE,
                        ):
                            fn()
                else:
                    ps_stack.close()

    nc.finalize()
    return nc


def _get_nc(with_bias=False):
    if with_bias not in _built:
        _built[with_bias] = _build(with_bias)
    return _built[with_bias]


def kernel(hidden_states, attention_mask, Wq, bq, Wk, bk, Wv, bv, Wo, bo):
    hidden_states = np.asarray(hidden_states, dtype=np.float32)
    Wq, Wk, Wv, Wo = (np.asarray(w, dtype=np.float32) for w in (Wq, Wk, Wv, Wo))
    bq, bk, bv, bo = (np.asarray(v, dtype=np.float32) for v in (bq, bk, bv, bo))

    with_bias = bool(np.any(bq) or np.any(bk) or np.any(bv))

    x8 = hidden_states.reshape(T, D) * 8.0
    # XT[p, pair, ko, t] = 8 * x[pair*1024 + t, ko*128 + p]
    xt = np.ascontiguousarray(
        x8.reshape(NPAIR, 1024, KO, 128).transpose(3, 0, 2, 1)
    ).astype(BF16)
    # fp8 copy of chunks 0..NF8-1 at the same 8x scale (Q/K DoubleRow part)
    xt8 = np.ascontiguousarray(
        np.clip(x8[:, : NF8 * 128], -F8MAX, F8MAX)
        .reshape(NPAIR, 1024, NF8, 128)
        .transpose(3, 0, 2, 1)
    ).astype(F8)

    # causal 0/1 mask for the single [128,128] chunk on the diagonal:
    # mask[p, f] = p <= f
    mask = (np.arange(128)[:, None] <= np.arange(128)[None, :]).astype(BF16)
    iden = np.eye(128, dtype=BF16)
    ones = np.ones((128, 512), dtype=BF16)

    in_maps = []
    d8 = NF8 * 128
    for c in range(NCORES):
        rows = slice(c * M, (c + 1) * M)
        # bf16 W*T chunks NF8.. at 512x scale (matches the fp8 part's scale)
        wqt = np.ascontiguousarray(
            (Wq[rows, d8:].T * 512.0).reshape(KOB, 128, M).transpose(1, 0, 2)
        ).astype(BF16)
        wkt = np.ascontiguousarray(
            (Wk[rows, d8:].T * 512.0).reshape(KOB, 128, M).transpose(1, 0, 2)
        ).astype(BF16)
        wqt8 = np.ascontiguousarray(
            np.clip(Wq[rows, :d8].T * 512.0, -F8MAX, F8MAX)
            .reshape(NF8, 128, M)
            .transpose(1, 0, 2)
        ).astype(F8)
        wkt8 = np.ascontiguousarray(
            np.clip(Wk[rows, :d8].T * 512.0, -F8MAX, F8MAX)
            .reshape(NF8, 128, M)
            .transpose(1, 0, 2)
        ).astype(F8)
        # Wv/8 cancels X's 8x so V lands at natural scale
        wvt = np.ascontiguousarray(
            (Wv[rows, :].T / 8.0).reshape(KO, 128, M).transpose(1, 0, 2)
        ).astype(BF16)
        # WOT[p, h, oc, j] = Wo[oc*128 + j, rows0 + h*128 + p]
        wot = np.ascontiguousarray(
            Wo[:, rows].reshape(KO, 128, HPC, 128).transpose(3, 2, 0, 1)
        ).astype(BF16)
        bias = np.stack(
            [bq[rows] * 4096.0, bk[rows] * 4096.0, bv[rows]]
        )[None].astype(BF16)
        in_maps.append(
            {
                "XT": xt,
                "XT8": xt8,
                "WQT": wqt,
                "WKT": wkt,
                "WQT8": wqt8,
                "WKT8": wkt8,
                "WVT": wvt,
                "WOT": wot,
                "BIAS": np.ascontiguousarray(bias),
                "MASK": mask,
                "IDEN": iden,
                "ONES": ones,
            }
        )

    res = run_bass_kernel_spmd(_get_nc(with_bias), in_maps, list(range(NCORES)))
    out = res.results[0]["OUT"].astype(np.float32)
    for c in range(1, NCORES):
        out += res.results[c]["OUT"].astype(np.float32)
    out = np.ascontiguousarray(out.transpose(0, 2, 1))
    out += bo
    return out

